# revision 1
# baseline (speedup 1.0000x reference)
"""Trainium2 Bass kernel for multi-head self-attention (B=8, N=1024, C=768, H=12).

Sharding: data-parallel over batch -- one batch element per NeuronCore (8 cores).
Each core computes the full attention for its batch element; no collectives.

Per-core dataflow:
  x [N,C] --PE transpose (f32r single-pass)--> xT [C,N]
  qkvT [3C,N] = w_qkvT.T @ xT            (f32r matmuls, stationary = w chunks)
  per head h:  ST[m,n] = k_h @ q_h^T     (f32r, K=64)
               ET = exp(0.125*ST) -> bf16 (one ACT op per [128,1024] PSUM tile;
                                          no max-subtraction: |scores| < ~3)
               OT_unnorm[d,n], den[n] = [v_h | 1].T @ ET   (bf16 PV, ones-column)
               OT[d,n] = OT_unnorm * bcast(den)^-1  (PE K=1 bcast + fast recip)
  out [N,C] = OT.T @ w_projT + b_proj    (f32r, bias folded in as K=1 matmul)

The PE instruction stream is hand-interleaved so that ST score matmuls (whose
PSUM recycling is gated on the scalar engine's exp throughput) are spaced out
by independent PE work (qkv matmuls, weight transposes, the previous head's
PV accumulation).  Without this the PE micro-stalls every ~1us, the HAM
activity monitor half-clocks the PE, and the whole kernel runs ~2x slower.
"""

import numpy as np

import concourse.bass as bass
import concourse.tile as tile
from concourse import bacc
from concourse import mybir
from concourse.bass_utils import run_bass_kernel_spmd
from concourse.masks import make_identity

N = 1024
C = 768
H = 12
D = 64
O3 = 3 * C  # 2304
NCORES = 8
SCALE = D**-0.5

F32 = mybir.dt.float32
F32R = mybir.dt.float32r
BF16 = mybir.dt.bfloat16
EXP = mybir.ActivationFunctionType.Exp

NT_N = N // 128  # 8 n-blocks
NT_C = C // 128  # 6 c-chunks
NPAIR = H // 2   # 6 head pairs


def build_bass():
    nc = bacc.Bacc("TRN2", target_bir_lowering=False, debug=False, num_devices=NCORES)

    x = nc.dram_tensor("x", [N, C], F32R, kind="ExternalInput").ap()
    w_qkv = nc.dram_tensor("w_qkv", [O3, C], F32R, kind="ExternalInput").ap()
    w_proj = nc.dram_tensor("w_proj", [C, C], F32R, kind="ExternalInput").ap()
    b_proj = nc.dram_tensor("b_proj", [1, C], F32R, kind="ExternalInput").ap()
    out = nc.dram_tensor("out", [N, C], F32, kind="ExternalOutput").ap()

    with tile.TileContext(nc) as tc:
        with (
            tc.tile_pool(name="singles", bufs=1) as singles,
            tc.tile_pool(name="xT", bufs=1) as p_xT,
            tc.tile_pool(name="ld", bufs=4) as p_ld,
            tc.tile_pool(name="wT", bufs=2) as p_wT,
            tc.tile_pool(name="wpT", bufs=1) as p_wpT,
            tc.tile_pool(name="qkvT", bufs=2) as p_qkvT,
            tc.tile_pool(name="et", bufs=16) as p_et,
            tc.tile_pool(name="vn", bufs=1) as p_vn,
            tc.tile_pool(name="OT", bufs=NT_C) as p_OT,
            tc.tile_pool(name="otmp", bufs=2) as p_otmp,
            tc.tile_pool(name="dn", bufs=4) as p_dn,
            tc.tile_pool(name="rcb", bufs=2) as p_rcb,
            tc.tile_pool(name="osb", bufs=2) as p_osb,
            # PSUM: 8 banks total
            tc.tile_pool(name="pp_s", bufs=2, space="PSUM") as pp_s,    # 2x2 banks
            tc.tile_pool(name="pp_sm", bufs=4, space="PSUM") as pp_sm,  # 4x1 bank
        ):
            # ---- setup ----
            identity = singles.tile([128, 128], F32, tag="identity")
            make_identity(nc, identity[:])
            identity_r = singles.tile([128, 128], F32R, tag="identity_r")
            nc.vector.tensor_copy(identity_r[:], identity[:])
            ones_f = singles.tile([128, 128], F32, tag="ones_f")
            nc.vector.memset(ones_f[:], 1.0)
            ones = singles.tile([128, 128], F32R, tag="ones")
            nc.vector.tensor_copy(ones[:], ones_f[:])
            ones_b = singles.tile([128, 1], BF16, tag="ones_b")
            nc.vector.tensor_copy(ones_b[:], ones_f[:, 0:1])
            b_row = singles.tile([1, C], F32R, tag="b_row")
            nc.sync.dma_start(b_row[:], b_proj)

            # persistent vn tile: 16 slots of [v_h block (64) | ones] = 65 cols
            vn = p_vn.tile([128, 16 * 65], BF16, tag="vn")
            ones_cols = bass.AP(
                tensor=vn.tensor, offset=vn.offset + 64, ap=[vn.ap[0], [65, 16], [1, 1]]
            )
            ones_rep = bass.AP(
                tensor=ones_b.tensor,
                offset=ones_b.offset,
                ap=[ones_b.ap[0], [0, 16], [1, 1]],
            )
            nc.vector.tensor_copy(ones_cols, ones_rep)

            def tpose_pair(dst_tensor_ap, dst_off, dst_stride, src, j0, ident, w=128):
                """Two PE transposes into one PSUM tile + one strided DVE copy."""
                pt = pp_sm.tile([128, 2 * w], F32R, tag="pp_sm")
                nc.tensor.transpose(pt[:, 0:w], src(j0), ident)
                nc.tensor.transpose(pt[:, w : 2 * w], src(j0 + 1), ident)
                dst = bass.AP(
                    tensor=dst_tensor_ap.tensor,
                    offset=dst_tensor_ap.offset + dst_off,
                    ap=[dst_tensor_ap.ap[0], [dst_stride, 2], [1, w]],
                )
                nc.vector.tensor_copy(
                    dst, pt[:].rearrange("p (two c) -> p two c", two=2)
                )

            # ---- phase X: x -> xT (single [128, 6*1024] f32r tile) ----
            xT = p_xT.tile([128, NT_C * N], F32R, tag="xT")
            for i in range(NT_N):
                xn = p_ld.tile([128, C], F32R, tag="ld")
                nc.sync.dma_start(xn[:], x[i * 128 : (i + 1) * 128, :])
                for j0 in range(0, NT_C, 2):
                    tpose_pair(
                        xT,
                        j0 * N + i * 128,
                        N,
                        lambda j, xn=xn: xn[:, j * 128 : (j + 1) * 128],
                        j0,
                        identity_r[:],
                    )

            # ---- wproj -> wpT (single [128, 6*768] tile) ----
            wpT = p_wpT.tile([128, NT_C * C], F32R, tag="wpT")
            for i in range(NT_C):
                wpn = p_ld.tile([128, C], F32R, tag="ld")
                nc.sync.dma_start(wpn[:], w_proj[i * 128 : (i + 1) * 128, :])
                for j0 in range(0, NT_C, 2):
                    tpose_pair(
                        wpT,
                        j0 * C + i * 128,
                        C,
                        lambda j, wpn=wpn: wpn[:, j * 128 : (j + 1) * 128],
                        j0,
                        identity_r[:],
                    )

            # ---- emission helpers for the interleaved pair pipeline ----
            OT = [
                p_OT.tile([128, N], F32R, tag="OT", name=f"OT{_}")
                for _ in range(NT_C)
            ]

            def load_wn(hp):
                """Prefetch the 3 natural w_qkv row-blocks for pair hp."""
                wns = []
                for part in range(3):
                    row0 = part * C + hp * 128
                    wn = p_ld.tile([128, C], F32R, tag="ld", name=f"wn{hp}_{part}")
                    nc.sync.dma_start(wn[:], w_qkv[row0 : row0 + 128, :])
                    wns.append(wn)
                return wns

            def emit_wTT_part(wT_t, wns, part):
                """Transpose one w part (3 pt-pairs) into wT tile columns."""
                wn = wns[part]
                for j0 in range(0, NT_C, 2):
                    tpose_pair(
                        wT_t,
                        j0 * 384 + part * 128,
                        384,
                        lambda j, wn=wn: wn[:, j * 128 : (j + 1) * 128],
                        j0,
                        identity_r[:],
                    )

            def emit_qkv_part(blk, wT_t, part):
                ps = pp_s.tile([128, 1024], F32, tag="pp_s")
                for nj in range(2):
                    nsl = slice(nj * 512, (nj + 1) * 512)
                    for j in range(NT_C):
                        nc.tensor.matmul(
                            ps[:, nsl],
                            wT_t[:, j * 384 + part * 128 : j * 384 + (part + 1) * 128],
                            xT[:, j * N + nj * 512 : j * N + nj * 512 + 512],
                            start=(j == 0),
                            stop=(j == NT_C - 1),
                        )
                nc.vector.tensor_copy(blk[:, part * N : (part + 1) * N], ps[:])

            def emit_vnT(blk, h2):
                isl = slice(h2 * 64, h2 * 64 + 64)
                vT = blk[isl, 2 * N : 3 * N]
                for t0 in range(0, NT_N, 2):
                    tpose_pair(
                        vn,
                        (h2 * 8 + t0) * 65,
                        65,
                        lambda t, vT=vT: vT[:, t * 128 : (t + 1) * 128],
                        t0,
                        identity_r[isl, isl],
                        w=64,
                    )

            def emit_st(blk, h2, t, ets):
                rsl = slice(h2 * 64, h2 * 64 + 64)
                qT = blk[rsl, 0:N]
                kT = blk[rsl, N : 2 * N]
                ps = pp_s.tile([128, 1024], F32, tag="pp_s")
                for nj in range(2):
                    nsl = slice(nj * 512, (nj + 1) * 512)
                    nc.tensor.matmul(
                        ps[:, nsl],
                        kT[:, t * 128 : (t + 1) * 128],
                        qT[:, nsl],
                        start=True,
                        stop=True,
                    )
                e = p_et.tile([128, N], BF16, tag="et")
                nc.scalar.activation(e[:], ps[:], EXP, scale=SCALE)
                ets.append(e)

            def emit_pv(h2, nj, ets, po, dns):
                nsl = slice(nj * 512, (nj + 1) * 512)
                p_ = pp_sm.tile([65, 512], F32, tag="pp_sm")
                po.append(p_)
                for t in range(NT_N):
                    nc.tensor.matmul(
                        p_[:],
                        vn[:, (h2 * 8 + t) * 65 : (h2 * 8 + t + 1) * 65],
                        ets[t][:, nsl],
                        start=(t == 0),
                        stop=(t == NT_N - 1),
                    )
                dn = p_dn.tile([65, 512], F32R, tag="dn")
                nc.scalar.copy(dn[64:65, :], p_[64:65, :])
                dns.append(dn)

            def emit_norm(hp, h2, OT_hp, ot_dst, po, dns):
                pbs = []
                for nj in range(2):
                    pb = pp_sm.tile([64, 512], F32, tag="pp_sm")
                    nc.tensor.matmul(
                        pb[:], ones[64:65, 0:64], dns[nj][64:65, :],
                        start=True, stop=True,
                    )
                    pbs.append(pb)
                for nj in range(2):
                    nsl = slice(nj * 512, (nj + 1) * 512)
                    rcb = p_rcb.tile([64, 512], F32, tag="rcb")
                    nc.vector.reciprocal_approx_fast(rcb[:], pbs[nj][:])
                    if h2 == 0:
                        nc.vector.tensor_mul(OT_hp[0:64, nsl], po[nj][0:64, :], rcb[:])
                    else:
                        nc.vector.tensor_mul(ot_dst[:, nsl], po[nj][0:64, :], rcb[:])

            # ---- steady-state pipeline over head pairs ----
            # qkv q/k of pair hp+1 are emitted in pair hp's TAIL, interleaved
            # around PV(h1,*): that is where the PE otherwise catches up with
            # the scalar engine's exp queue and stalls (HAM then half-clocks).
            wns = load_wn(0)
            wT_cur = p_wT.tile([128, NT_C * 384], F32R, tag="wT", name="wT0")
            for part in range(3):
                emit_wTT_part(wT_cur, wns, part)

            for hp in range(NPAIR):
                blk = p_qkvT.tile([128, 3 * N], F32R, tag="qkvT")
                if hp + 1 < NPAIR:
                    wns_next = load_wn(hp + 1)
                    wT_next = p_wT.tile(
                        [128, NT_C * 384], F32R, tag="wT", name=f"wT{hp + 1}"
                    )
                OT_hp = OT[hp]
                ot_dst = p_otmp.tile([64, N], F32R, tag="otmp")

                emit_qkv_part(blk, wT_cur, 0)  # q
                emit_qkv_part(blk, wT_cur, 1)  # k

                # ST(h0) stretch, spaced by independent PE filler work
                et0, et1 = [], []
                emit_st(blk, 0, 0, et0)
                emit_qkv_part(blk, wT_cur, 2)  # v
                emit_st(blk, 0, 1, et0)
                for t in (2, 3, 4):
                    if hp + 1 < NPAIR:
                        emit_wTT_part(wT_next, wns_next, t - 2)
                    emit_st(blk, 0, t, et0)
                emit_vnT(blk, 0)
                emit_st(blk, 0, 5, et0)
                emit_vnT(blk, 1)
                emit_st(blk, 0, 6, et0)
                emit_st(blk, 0, 7, et0)

                # ST(h1) stretch, spaced by PV(h0) + its normalization
                po0, dn0 = [], []
                emit_st(blk, 1, 0, et1)
                emit_pv(0, 0, et0, po0, dn0)
                emit_st(blk, 1, 1, et1)
                emit_st(blk, 1, 2, et1)
                emit_pv(0, 1, et0, po0, dn0)
                emit_st(blk, 1, 3, et1)
                emit_st(blk, 1, 4, et1)
                emit_norm(hp, 0, OT_hp, ot_dst, po0, dn0)
                emit_st(blk, 1, 5, et1)
                emit_st(blk, 1, 6, et1)
                emit_st(blk, 1, 7, et1)

                po1, dn1 = [], []
                emit_pv(1, 0, et1, po1, dn1)
                emit_pv(1, 1, et1, po1, dn1)
                emit_norm(hp, 1, OT_hp, ot_dst, po1, dn1)
                # partition shift 0:64 -> 64:128 via SBUF-to-SBUF DMA
                nc.sync.dma_start(OT_hp[64:128, :], ot_dst[:])

                if hp + 1 < NPAIR:
                    wT_cur = wT_next

            # ---- phase C: proj (dense) ----
            for i in range(NT_N):
                ps = pp_s.tile([128, 1024], F32, tag="pp_s")
                for osl in (slice(0, 512), slice(512, 768)):
                    for j in range(NT_C):
                        nc.tensor.matmul(
                            ps[:, osl],
                            OT[j][:, i * 128 : (i + 1) * 128],
                            wpT[:, j * C + osl.start : j * C + osl.stop],
                            start=(j == 0),
                            stop=False,
                        )
                    nc.tensor.matmul(
                        ps[:, osl],
                        ones[0:1, 0:128],
                        b_row[:, osl],
                        start=False,
                        stop=True,
                    )
                osb = p_osb.tile([128, C], F32, tag="osb")
                nc.vector.tensor_copy(osb[:], ps[:, 0:C])
                nc.sync.dma_start(out[i * 128 : (i + 1) * 128, :], osb[:])

    nc.compile()
    return nc


_NC_CACHE = None


def kernel(x, w_qkv, w_proj, b_proj):
    global _NC_CACHE
    if _NC_CACHE is None:
        _NC_CACHE = build_bass()
    nc = _NC_CACHE

    x = np.ascontiguousarray(np.asarray(x, dtype=np.float32))
    w_qkv = np.ascontiguousarray(np.asarray(w_qkv, dtype=np.float32))
    w_proj = np.ascontiguousarray(np.asarray(w_proj, dtype=np.float32))
    b_row = np.ascontiguousarray(
        np.asarray(b_proj, dtype=np.float32).reshape(1, C)
    )

    in_maps = [
        {"x": x[b], "w_qkv": w_qkv, "w_proj": w_proj, "b_proj": b_row}
        for b in range(NCORES)
    ]
    res = run_bass_kernel_spmd(nc, in_maps, list(range(NCORES)))
    return np.stack([res.results[b]["out"] for b in range(NCORES)], axis=0)



# revision 5
# speedup vs baseline: 1.6406x; 1.6406x over previous
"""Trainium2 Bass kernel for multi-head self-attention (B=8, N=1024, C=768, H=12).

Sharding: data-parallel over batch -- one batch element per NeuronCore (8 cores).

Key design points (v2, rewritten from the f32r baseline):
  * All operand transposes are done on the HOST (numpy) -- the device kernel
    contains ZERO PE-transpose instructions.  PE transposes don't count as
    PE-busy for the HAM clock gate, so the baseline's 288 transposes both
    cost ~80us of PE slices and kept re-throttling the PE to 1.2 GHz.
  * Everything is bf16 (1.0 cycles/row, FWL-eligible weight loads, half DMA).
  * v is computed in NATURAL [n, c] layout via an xT-stationary GEMM
    (out[n,vc] = xT[k,n].T @ wvT[k,vc]) so the per-head [keys, 64|1] PV
    stationary tiles are built with cheap DVE copies instead of PE transposes.
  * ST score matmuls have K=64: the two heads of a pair sit on SBUF
    partitions 0:64 / 64:128, so their matmuls land on disjoint PE row
    groups (tile_position (0,0) vs (64,0)) and execute CONCURRENTLY when
    issued back-to-back (~2x effective ST throughput).
  * Per head pair the PE work (qk GEMM for next pair + ST + PV + norm
    broadcasts ~16.2us) matches the ACT exp work (16 x [128,1024] exp =
    16.4us), so both engines stay ~100% busy and the HAM stays at K=8/8.

Per-core dataflow:
  qkT  [256, N] per pair = wqkT_pair.T @ xT   (bf16, stationary = w slices)
  v    [N, C]  = xT.T @ wvT                    (bf16, natural layout)
  per head h:  ST[m,n] = k_h @ q_h^T           (bf16, K=64, row-tiled pairs)
               ET = exp(0.125*ST) -> bf16      (one ACT op per [128,1024])
               PV: [v_h | 1].T @ ET -> [65, n] unnormalized + denominator
               OT[d,n] = PV[0:64] * bcast(den)^-1   (K=1 PE bcast + DVE)
  out [N, C] = OT.T @ wprojT + b_proj          (bf16, bias as K=1 f32r matmul)
"""

import numpy as np
import ml_dtypes

import concourse.bass as bass
import concourse.tile as tile
from concourse import bacc
from concourse import mybir
from concourse.bass_utils import run_bass_kernel_spmd

N = 1024
C = 768
H = 12
D = 64
NCORES = 8
SCALE = D**-0.5

F32 = mybir.dt.float32
F32R = mybir.dt.float32r
BF16 = mybir.dt.bfloat16
EXP = mybir.ActivationFunctionType.Exp

NT_N = N // 128  # 8 n-blocks / key tiles
NT_C = C // 128  # 6 k-chunks
NPAIR = H // 2   # 6 head pairs

BF = ml_dtypes.bfloat16


def build_bass():
    nc = bacc.Bacc("TRN2", target_bir_lowering=False, debug=False, num_devices=NCORES)

    # host-pretransposed inputs
    xT_d = nc.dram_tensor("xT", [C, N], BF16, kind="ExternalInput").ap()
    # per-pair packed [q_pair(128) | k_pair(128)] columns: [NPAIR*768, 256]
    wqk_d = nc.dram_tensor("wqk", [NPAIR * C, 256], BF16, kind="ExternalInput").ap()
    wv_d = nc.dram_tensor("wv", [C, C], BF16, kind="ExternalInput").ap()  # w_v^T
    wp_d = nc.dram_tensor("wp", [C, C], BF16, kind="ExternalInput").ap()  # w_proj^T
    b_d = nc.dram_tensor("b_proj", [1, C], F32R, kind="ExternalInput").ap()
    out_d = nc.dram_tensor("out", [N, C], F32, kind="ExternalOutput").ap()

    with tile.TileContext(nc) as tc:
        with (
            tc.tile_pool(name="singles", bufs=1) as singles,
            tc.tile_pool(name="wqk", bufs=2) as p_wqk,
            tc.tile_pool(name="qk", bufs=2) as p_qk,
            tc.tile_pool(name="et", bufs=32) as p_et,
            tc.tile_pool(name="OT", bufs=NPAIR) as p_OT,
            tc.tile_pool(name="pvs", bufs=4) as p_pvs,
            tc.tile_pool(name="rcb", bufs=4) as p_rcb,
            tc.tile_pool(name="ott", bufs=2) as p_ott,
            tc.tile_pool(name="osb", bufs=2) as p_osb,
            # PSUM: 8 banks total
            tc.tile_pool(name="pp_st", bufs=2, space="PSUM") as pp_st,  # 2x2 banks
            tc.tile_pool(name="pp_sm", bufs=4, space="PSUM") as pp_sm,  # 4x1 bank
        ):
            # ---------------- setup ----------------
            ones_f = singles.tile([128, 128], F32, tag="ones_f")
            nc.vector.memset(ones_f[:], 1.0)
            ones_r = singles.tile([128, 128], F32R, tag="ones_r")
            nc.vector.tensor_copy(ones_r[:], ones_f[:])
            ones_b = singles.tile([128, 1], BF16, tag="ones_b")
            nc.vector.tensor_copy(ones_b[:], ones_f[:, 0:1])
            b_row = singles.tile([1, C], F32R, tag="b_row")
            nc.sync.dma_start(b_row[:], b_d)

            # persistent SBUF planes
            xT = singles.tile([128, NT_C * N], BF16, tag="xT")      # [k, n] chunks
            wv = singles.tile([128, NT_C * C], BF16, tag="wv")      # [k, vc] chunks
            wp = singles.tile([128, NT_C * C], BF16, tag="wp")      # [cj, oc] chunks
            vnat = singles.tile([128, NT_N * C], BF16, tag="vnat")  # [n, vc] blocks
            # per (h, t) PV stationary slots [keys, v(64) | ones]
            vn = singles.tile([128, H * NT_N * 65], BF16, tag="vn")

            # ones column of every vn slot
            ones_cols = bass.AP(
                tensor=vn.tensor,
                offset=vn.offset + 64,
                ap=[vn.ap[0], [65, H * NT_N], [1, 1]],
            )
            ones_rep = bass.AP(
                tensor=ones_b.tensor,
                offset=ones_b.offset,
                ap=[ones_b.ap[0], [0, H * NT_N], [1, 1]],
            )
            nc.vector.tensor_copy(ones_cols, ones_rep)

            # ---------------- input DMAs ----------------
            for kc in range(NT_C):
                nc.sync.dma_start(
                    xT[:, kc * N : (kc + 1) * N], xT_d[kc * 128 : (kc + 1) * 128, :]
                )

            def dma_wqk(hp):
                t = p_wqk.tile([128, NT_C * 256], BF16, tag="wqk", name=f"wqk{hp}")
                for kc in range(NT_C):
                    nc.sync.dma_start(
                        t[:, kc * 256 : (kc + 1) * 256],
                        wqk_d[hp * C + kc * 128 : hp * C + (kc + 1) * 128, :],
                    )
                return t

            wqk_cur = dma_wqk(0)
            for kc in range(NT_C):
                nc.sync.dma_start(
                    wv[:, kc * C : (kc + 1) * C], wv_d[kc * 128 : (kc + 1) * 128, :]
                )
            for kc in range(NT_C):
                nc.sync.dma_start(
                    wp[:, kc * C : (kc + 1) * C], wp_d[kc * 128 : (kc + 1) * 128, :]
                )

            # ---------------- emission helpers ----------------
            def emit_qk_first(wqk_t):
                """Pair-0 qk GEMM, k-chunk outer so PE starts as DMA lands."""
                qk_sb = p_qk.tile([128, 2 * N], BF16, tag="qk", name="qk0")
                ps = {}
                for part in range(2):
                    for nj in range(2):
                        ps[part, nj] = pp_sm.tile(
                            [128, 512], F32, tag="pp_sm", name=f"qkps{part}{nj}"
                        )
                for kc in range(NT_C):
                    for part in range(2):
                        for nj in range(2):
                            nc.tensor.matmul(
                                ps[part, nj][:],
                                wqk_t[:, kc * 256 + part * 128 : kc * 256 + (part + 1) * 128],
                                xT[:, kc * N + nj * 512 : kc * N + nj * 512 + 512],
                                start=(kc == 0),
                                stop=(kc == NT_C - 1),
                            )
                for part in range(2):
                    for nj in range(2):
                        nc.vector.tensor_copy(
                            qk_sb[:, part * N + nj * 512 : part * N + nj * 512 + 512],
                            ps[part, nj][:],
                        )
                return qk_sb

            def emit_qk_group(qk_sb, wqk_t, part, nj):
                """One (part, nj) quarter of a pair's qk GEMM: 6 MMs + copy."""
                ps = pp_sm.tile([128, 512], F32, tag="pp_sm")
                for kc in range(NT_C):
                    nc.tensor.matmul(
                        ps[:],
                        wqk_t[:, kc * 256 + part * 128 : kc * 256 + (part + 1) * 128],
                        xT[:, kc * N + nj * 512 : kc * N + nj * 512 + 512],
                        start=(kc == 0),
                        stop=(kc == NT_C - 1),
                    )
                nc.vector.tensor_copy(
                    qk_sb[:, part * N + nj * 512 : part * N + nj * 512 + 512], ps[:]
                )

            def emit_v_group(nb):
                """v GEMM for one n-block: v_nat[nb] = xT[:, nb].T @ wvT."""
                ps1 = pp_sm.tile([128, 512], F32, tag="pp_sm")
                ps2 = pp_sm.tile([128, 256], F32, tag="pp_sm")
                for kc in range(NT_C):
                    lhsT = xT[:, kc * N + nb * 128 : kc * N + (nb + 1) * 128]
                    nc.tensor.matmul(
                        ps1[:], lhsT, wv[:, kc * C : kc * C + 512],
                        start=(kc == 0), stop=(kc == NT_C - 1),
                    )
                    nc.tensor.matmul(
                        ps2[:], lhsT, wv[:, kc * C + 512 : kc * C + 768],
                        start=(kc == 0), stop=(kc == NT_C - 1),
                    )
                nc.vector.tensor_copy(vnat[:, nb * C : nb * C + 512], ps1[:])
                nc.vector.tensor_copy(vnat[:, nb * C + 512 : nb * C + 768], ps2[:])

            def emit_vn_copies(h, t):
                """Fill vn slot (h, t) from v_nat block t (ones col pre-set)."""
                s = (h * NT_N + t) * 65
                nc.vector.tensor_copy(
                    vn[:, s : s + 64], vnat[:, t * C + h * 64 : t * C + (h + 1) * 64]
                )

            def emit_st(qk_sb, t, ets):
                """Row-tiled concurrent ST pair for heads h0 (rows 0:64) and h1."""
                pss = []
                for h2 in range(2):
                    ps = pp_st.tile([128, N], F32, tag="pp_st")
                    pss.append(ps)
                for nj in range(2):
                    nsl = slice(nj * 512, (nj + 1) * 512)
                    for h2 in range(2):
                        rsl = slice(h2 * 64, h2 * 64 + 64)
                        nc.tensor.matmul(
                            pss[h2][:, nsl],
                            qk_sb[rsl, N + t * 128 : N + (t + 1) * 128],
                            qk_sb[rsl, nsl],
                            start=True,
                            stop=True,
                        )
                for h2 in range(2):
                    e = p_et.tile([128, N], BF16, tag="et")
                    nc.scalar.activation(e[:], pss[h2][:], EXP, scale=SCALE)
                    ets[h2].append(e)

            def emit_pv(h, nj, ets, po):
                """PV for one (head, n-half): accumulate 8 key tiles, M=65."""
                p_ = pp_sm.tile([65, 512], F32, tag="pp_sm")
                po.append(p_)
                nsl = slice(nj * 512, (nj + 1) * 512)
                for t in range(NT_N):
                    s = (h * NT_N + t) * 65
                    nc.tensor.matmul(
                        p_[:],
                        vn[:, s : s + 65],
                        ets[t][:, nsl],
                        start=(t == 0),
                        stop=(t == NT_N - 1),
                    )

            def emit_pv_copy(po, pvs):
                """Move PV psum (unnorm + den row) to SBUF, freeing the bank."""
                for nj in range(2):
                    pv = p_pvs.tile([65, 512], F32R, tag="pvs")
                    nc.vector.tensor_copy(pv[:], po[nj][:])
                    pvs.append(pv)

            def emit_bcast(pvs, pbs):
                """K=1 PE matmul broadcasting den across 64 partitions."""
                for nj in range(2):
                    pb = pp_sm.tile([64, 512], F32, tag="pp_sm")
                    nc.tensor.matmul(
                        pb[:], ones_r[64:65, 0:64], pvs[nj][64:65, :],
                        start=True, stop=True,
                    )
                    pbs.append(pb)

            def emit_norm(h2, OT_hp, ot_tmp, pvs, pbs):
                for nj in range(2):
                    nsl = slice(nj * 512, (nj + 1) * 512)
                    rcb = p_rcb.tile([64, 512], F32, tag="rcb")
                    nc.vector.reciprocal_approx_fast(rcb[:], pbs[nj][:])
                    dst = OT_hp[0:64, nsl] if h2 == 0 else ot_tmp[:, nsl]
                    nc.vector.tensor_mul(dst, pvs[nj][0:64, :], rcb[:])

            # ---------------- pair 0 qk GEMM (DMA-pipelined) ----------------
            qk_cur = emit_qk_first(wqk_cur)

            # ---------------- steady-state pair loop ----------------
            ets_prev = None   # [h2] -> list of 8 et tiles, previous pair
            hp_prev = None
            OT = [
                p_OT.tile([128, N], BF16, tag="OT", name=f"OT{j}")
                for j in range(NPAIR)
            ]

            for hp in range(NPAIR):
                if hp + 1 < NPAIR:
                    wqk_next = dma_wqk(hp + 1)
                    qk_next = p_qk.tile([128, 2 * N], BF16, tag="qk", name=f"qk{hp+1}")
                ets = [[], []]
                # filler state for PV(hp-1) + norm
                po_h = [[], []]
                pvs_h = [[], []]
                pbs_h = [[], []]
                if hp_prev is not None:
                    OT_prev = OT[hp_prev]
                    ot_tmp = p_ott.tile([64, N], BF16, tag="ott")

                for t in range(NT_N):
                    emit_st(qk_cur, t, ets)
                    if hp == 0:
                        # pair 0 fillers: v GEMM + vn assembly + qk GEMM pair 1
                        emit_v_group(t)
                        for h in (0, 1):  # vn slots for pair 0's heads
                            emit_vn_copies(h, t)
                        if t in (2, 3, 6, 7):
                            part, nj = {2: (0, 0), 3: (0, 1), 6: (1, 0), 7: (1, 1)}[t]
                            emit_qk_group(qk_next, wqk_next, part, nj)
                    else:
                        h_lo = 2 * hp_prev
                        if t == 0:
                            emit_pv(h_lo, 0, ets_prev[0], po_h[0])
                        elif t == 1:
                            emit_pv(h_lo, 1, ets_prev[0], po_h[0])
                        elif t == 2:
                            emit_pv_copy(po_h[0], pvs_h[0])
                            emit_bcast(pvs_h[0], pbs_h[0])
                            if hp + 1 < NPAIR:
                                emit_qk_group(qk_next, wqk_next, 0, 0)
                        elif t == 3:
                            emit_norm(0, OT_prev, None, pvs_h[0], pbs_h[0])
                            if hp + 1 < NPAIR:
                                emit_qk_group(qk_next, wqk_next, 0, 1)
                        elif t == 4:
                            emit_pv(h_lo + 1, 0, ets_prev[1], po_h[1])
                        elif t == 5:
                            emit_pv(h_lo + 1, 1, ets_prev[1], po_h[1])
                        elif t == 6:
                            emit_pv_copy(po_h[1], pvs_h[1])
                            emit_bcast(pvs_h[1], pbs_h[1])
                            if hp + 1 < NPAIR:
                                emit_qk_group(qk_next, wqk_next, 1, 0)
                        elif t == 7:
                            emit_norm(1, OT_prev, ot_tmp, pvs_h[1], pbs_h[1])
                            nc.sync.dma_start(OT_prev[64:128, :], ot_tmp[:])
                            if hp + 1 < NPAIR:
                                emit_qk_group(qk_next, wqk_next, 1, 1)
                        # vn slots for this pair's heads (needed by PV at hp+1)
                        for h in (2 * hp, 2 * hp + 1):
                            emit_vn_copies(h, t)

                ets_prev = ets
                hp_prev = hp
                if hp + 1 < NPAIR:
                    qk_cur = qk_next
                    wqk_cur = wqk_next

            # ---------------- tail: PV + norm of last pair ----------------
            h_lo = 2 * hp_prev
            OT_prev = OT[hp_prev]
            ot_tmp = p_ott.tile([64, N], BF16, tag="ott")
            po_h = [[], []]
            pvs_h = [[], []]
            pbs_h = [[], []]
            for h2 in range(2):
                emit_pv(h_lo + h2, 0, ets_prev[h2], po_h[h2])
                emit_pv(h_lo + h2, 1, ets_prev[h2], po_h[h2])
                emit_pv_copy(po_h[h2], pvs_h[h2])
                emit_bcast(pvs_h[h2], pbs_h[h2])
                emit_norm(h2, OT_prev, ot_tmp, pvs_h[h2], pbs_h[h2])
            nc.sync.dma_start(OT_prev[64:128, :], ot_tmp[:])

            # ---------------- proj ----------------
            for nb in range(NT_N):
                ps = pp_st.tile([128, N], F32, tag="pp_st")
                for osl in (slice(0, 512), slice(512, 768)):
                    for j in range(NPAIR):
                        nc.tensor.matmul(
                            ps[:, osl],
                            OT[j][:, nb * 128 : (nb + 1) * 128],
                            wp[:, j * C + osl.start : j * C + osl.stop],
                            start=(j == 0),
                            stop=False,
                        )
                    nc.tensor.matmul(
                        ps[:, osl],
                        ones_r[0:1, 0:128],
                        b_row[:, osl],
                        start=False,
                        stop=True,
                    )
                osb = p_osb.tile([128, C], F32, tag="osb")
                nc.vector.tensor_copy(osb[:], ps[:, 0:C])
                nc.sync.dma_start(out_d[nb * 128 : (nb + 1) * 128, :], osb[:])

    nc.compile()
    return nc


_NC_CACHE = None


def _prep_inputs(x, w_qkv, w_proj, b_proj):
    x = np.asarray(x, dtype=np.float32)
    w_qkv = np.asarray(w_qkv, dtype=np.float32)
    w_proj = np.asarray(w_proj, dtype=np.float32)
    b_row = np.ascontiguousarray(
        np.asarray(b_proj, dtype=np.float32).reshape(1, C)
    )

    # per-pair packed [768, 256] blocks: cols 0:128 = q rows of the pair
    # transposed, cols 128:256 = k rows of the pair transposed
    wqk_blocks = []
    for hp in range(NPAIR):
        qb = w_qkv[hp * 128 : (hp + 1) * 128, :]          # [128, 768]
        kb = w_qkv[C + hp * 128 : C + (hp + 1) * 128, :]  # [128, 768]
        wqk_blocks.append(np.concatenate([qb.T, kb.T], axis=1))  # [768, 256]
    wqk = np.ascontiguousarray(np.concatenate(wqk_blocks, axis=0)).astype(BF)
    wv = np.ascontiguousarray(w_qkv[2 * C :, :].T).astype(BF)   # [768, 768]
    wp = np.ascontiguousarray(w_proj.T).astype(BF)              # [768, 768]
    xTs = [np.ascontiguousarray(x[b].T).astype(BF) for b in range(NCORES)]
    return xTs, wqk, wv, wp, b_row


def kernel(x, w_qkv, w_proj, b_proj):
    global _NC_CACHE
    if _NC_CACHE is None:
        _NC_CACHE = build_bass()
    nc = _NC_CACHE

    xTs, wqk, wv, wp, b_row = _prep_inputs(x, w_qkv, w_proj, b_proj)
    in_maps = [
        {"xT": xTs[b], "wqk": wqk, "wv": wv, "wp": wp, "b_proj": b_row}
        for b in range(NCORES)
    ]
    res = run_bass_kernel_spmd(nc, in_maps, list(range(NCORES)))
    return np.stack([res.results[b]["out"] for b in range(NCORES)], axis=0)
